# revision 1
# baseline (speedup 1.0000x reference)
"""SAM-style windowed attention w/ decomposed rel-pos bias on 8 trn2 NeuronCores.

Sharding: pure data-parallel over batch B=8 -> 1 batch element per core
(12 heads each); projection weights + rel-pos tables replicated. No
collectives needed; outputs are gathered by stacking the per-device
results back into the full (8, 32, 32, 768) tensor.
"""
import numpy as np
import jax
import jax.numpy as jnp
from functools import partial

NUM_HEADS = 12
B, H, W, DIM = 8, 32, 32, 768
HEAD_DIM = DIM // NUM_HEADS  # 64
N = H * W  # 1024


def _attn_one(x, qkv_w, qkv_b, proj_w, proj_b, Rh, Rw):
    """x: (H, W, dim) one batch element. Rh: (H, H, hd), Rw: (W, W, hd)."""
    scale = HEAD_DIM ** (-0.5)
    qkv = x.reshape(N, DIM) @ qkv_w + qkv_b                      # (N, 3*dim)
    qkv = qkv.reshape(N, 3, NUM_HEADS, HEAD_DIM)
    qkv = qkv.transpose(1, 2, 0, 3)                              # (3, h, N, hd)
    q, k, v = qkv[0], qkv[1], qkv[2]                             # (h, N, hd)

    attn = jnp.einsum("bnd,bmd->bnm", q * scale, k)              # (h, N, N)

    r_q = q.reshape(NUM_HEADS, H, W, HEAD_DIM)
    rel_h = jnp.einsum("bhwc,hkc->bhwk", r_q, Rh)                # (h,H,W,H)
    rel_w = jnp.einsum("bhwc,wkc->bhwk", r_q, Rw)                # (h,H,W,W)
    attn = (attn.reshape(NUM_HEADS, H, W, H, W)
            + rel_h[:, :, :, :, None]
            + rel_w[:, :, :, None, :]).reshape(NUM_HEADS, N, N)

    attn = jax.nn.softmax(attn, axis=-1)
    out = jnp.einsum("bnm,bmd->bnd", attn, v)                    # (h, N, hd)
    out = out.reshape(NUM_HEADS, H, W, HEAD_DIM).transpose(1, 2, 0, 3)
    out = out.reshape(H, W, DIM)
    return out @ proj_w + proj_b


@partial(jax.pmap, in_axes=(0, None, None, None, None, None, None))
def _run_sharded(x, qkv_w, qkv_b, proj_w, proj_b, Rh, Rw):
    return _attn_one(x, qkv_w, qkv_b, proj_w, proj_b, Rh, Rw)


def _get_rel(size, table):
    idx = np.arange(size)[:, None] - np.arange(size)[None, :] + (size - 1)
    return table[idx]  # (size, size, hd)


def kernel(x, qkv_w, qkv_b, proj_w, proj_b, rel_pos_h, rel_pos_w):
    x = np.asarray(x, np.float32)
    # host-side: resolve the tiny static index gathers of the rel-pos tables
    Rh = _get_rel(H, np.asarray(rel_pos_h, np.float32))  # (H, H, hd)
    Rw = _get_rel(W, np.asarray(rel_pos_w, np.float32))  # (W, W, hd)
    out = _run_sharded(
        x,  # (8, H, W, dim): leading axis == 8 devices
        np.asarray(qkv_w, np.float32), np.asarray(qkv_b, np.float32),
        np.asarray(proj_w, np.float32), np.asarray(proj_b, np.float32),
        Rh, Rw,
    )
    return np.asarray(out).astype(np.float32)  # (8, H, W, dim)


# revision 2
# speedup vs baseline: 1.0107x; 1.0107x over previous
"""SAM-style windowed attention w/ decomposed rel-pos bias on 8 trn2 NeuronCores.

Sharding: pure data-parallel over batch B=8 -> 1 batch element per core
(12 heads each); projection weights + rel-pos tables replicated. No
collectives needed; outputs are gathered by stacking the per-device
results back into the full (8, 32, 32, 768) tensor.
"""
import numpy as np
import jax
import jax.numpy as jnp
from functools import partial

NUM_HEADS = 12
B, H, W, DIM = 8, 32, 32, 768
HEAD_DIM = DIM // NUM_HEADS  # 64
N = H * W  # 1024


def _attn_one(x, qkv_w, qkv_b, proj_w, proj_b, Rh, Rw):
    """x: (H, W, dim) one batch element. Rh: (H, H, hd), Rw: (W, W, hd).

    Matmuls run in bf16 (f32 accumulate) for TensorEngine rate; softmax,
    bias adds, and all reductions stay f32.
    """
    bf = jnp.bfloat16
    f32 = jnp.float32
    scale = HEAD_DIM ** (-0.5)
    xb = x.reshape(N, DIM).astype(bf)
    qkv = jnp.matmul(xb, qkv_w.astype(bf),
                     preferred_element_type=f32) + qkv_b         # (N, 3*dim)
    qkv = qkv.reshape(N, 3, NUM_HEADS, HEAD_DIM)
    qkv = qkv.transpose(1, 2, 0, 3)                              # (3, h, N, hd)
    q, k, v = qkv[0], qkv[1], qkv[2]                             # (h, N, hd)

    attn = jnp.einsum("bnd,bmd->bnm", (q * scale).astype(bf),
                      k.astype(bf), preferred_element_type=f32)  # (h, N, N)

    r_q = q.reshape(NUM_HEADS, H, W, HEAD_DIM).astype(bf)
    rel_h = jnp.einsum("bhwc,hkc->bhwk", r_q, Rh.astype(bf),
                       preferred_element_type=f32)               # (h,H,W,H)
    rel_w = jnp.einsum("bhwc,wkc->bhwk", r_q, Rw.astype(bf),
                       preferred_element_type=f32)               # (h,H,W,W)
    attn = (attn.reshape(NUM_HEADS, H, W, H, W)
            + rel_h[:, :, :, :, None]
            + rel_w[:, :, :, None, :]).reshape(NUM_HEADS, N, N)

    attn = jax.nn.softmax(attn, axis=-1)
    out = jnp.einsum("bnm,bmd->bnd", attn.astype(bf), v.astype(bf),
                     preferred_element_type=f32)                 # (h, N, hd)
    out = out.reshape(NUM_HEADS, H, W, HEAD_DIM).transpose(1, 2, 0, 3)
    out = out.reshape(H, W, DIM)
    return jnp.matmul(out.astype(bf), proj_w.astype(bf),
                      preferred_element_type=f32) + proj_b


@partial(jax.pmap, in_axes=(0, None, None, None, None, None, None))
def _run_sharded(x, qkv_w, qkv_b, proj_w, proj_b, Rh, Rw):
    return _attn_one(x, qkv_w, qkv_b, proj_w, proj_b, Rh, Rw)


def _get_rel(size, table):
    idx = np.arange(size)[:, None] - np.arange(size)[None, :] + (size - 1)
    return table[idx]  # (size, size, hd)


def kernel(x, qkv_w, qkv_b, proj_w, proj_b, rel_pos_h, rel_pos_w):
    x = np.asarray(x, np.float32)
    # host-side: resolve the tiny static index gathers of the rel-pos tables
    Rh = _get_rel(H, np.asarray(rel_pos_h, np.float32))  # (H, H, hd)
    Rw = _get_rel(W, np.asarray(rel_pos_w, np.float32))  # (W, W, hd)
    out = _run_sharded(
        x,  # (8, H, W, dim): leading axis == 8 devices
        np.asarray(qkv_w, np.float32), np.asarray(qkv_b, np.float32),
        np.asarray(proj_w, np.float32), np.asarray(proj_b, np.float32),
        Rh, Rw,
    )
    return np.asarray(out).astype(np.float32)  # (8, H, W, dim)
